# revision 1
# baseline (speedup 1.0000x reference)
"""DeepseekVL2 MoE gate (sigmoid + grouped top-k routing) on 8 trn2 cores.

Contract: kernel(**inputs) takes the FULL unsharded inputs
  hidden_states [4, 4096, 7168] f32, weight [256, 7168] f32,
  e_score_correction_bias [256] f32
and returns (topk_idx [16384, 8] int32, topk_weight [16384, 8] f32),
matching reference jax semantics.

Strategy:
  - Data parallel: 16384 tokens -> 2048 per core x 8 cores.
  - Gating GEMM at full PE rate: x and w.T are split on host into fp16
    hi/lo pairs; logits = xh@wh + xh@wl + xl@wh accumulated in fp32 PSUM
    (~1e-6 abs logit error, below fp32 reorder noise; native fp32 matmul
    would run at 1/4 PE rate). w is pre-scaled by 1024 so w_lo stays in
    fp16 normal range; the 1/1024 is folded into the sigmoid's scale.
  - Routing per 128-token tile entirely on-chip with DVE max8 /
    max_index / match_replace ops (tie semantics match jax top_k).
"""

import os
import numpy as np

import concourse.bacc as bacc
import concourse.bass as bass
import concourse.mybir as mybir
from concourse.bass_utils import run_bass_kernel_spmd
from concourse.tile import TileContext

F16 = mybir.dt.float16
F32 = mybir.dt.float32
U32 = mybir.dt.uint32
I32 = mybir.dt.int32

N_CORES = 8
T_FULL = 16384
T_CORE = T_FULL // N_CORES          # 2048
H = 7168
E = 256
KT = H // 128                        # 56 contraction tiles
N_TILES = T_CORE // 128              # 16 token tiles per core
N_GROUP = 8
GROUP_SIZE = E // N_GROUP            # 32
TOPK_GROUP = 4
TOP_K = 8
ROUTED_SCALING = 2.5
W_SCALE = 1024.0                     # keeps w_lo fp16-normal
NEG_BIG = -1.0e30


def _build_nc():
    nc = bacc.Bacc(
        "TRN2",
        target_bir_lowering=False,
        debug=False,
        num_devices=N_CORES,
    )

    # x split tensors arrive pre-shuffled to SBUF layout: [p, tile, k, t] so
    # each partition's per-tile data is one contiguous 14KB run (fast DMA)
    xh_d = nc.dram_tensor("xh", [128, N_TILES, KT, 128], F16, kind="ExternalInput").ap()
    xl_d = nc.dram_tensor("xl", [128, N_TILES, KT, 128], F16, kind="ExternalInput").ap()
    # wcat = [w_hi | w_lo] along the expert dim: one N=512 matmul computes
    # xh@wh and xh@wl side by side in PSUM. Pre-shuffled on host to
    # [p, chunk, k, e] so each chunk DMA is 128 contiguous 7KB runs
    # (vs 896x1KB descriptors from the natural [H, 2E] layout)
    wcat_d = nc.dram_tensor(
        "wcat", [128, 8, 7, 2 * E], F16, kind="ExternalInput"
    ).ap()
    bias_d = nc.dram_tensor("biasb", [128, E], F32, kind="ExternalInput").ap()
    idx_d = nc.dram_tensor("out_idx", [T_CORE, TOP_K], I32, kind="ExternalOutput").ap()
    w_d = nc.dram_tensor("out_w", [T_CORE, TOP_K], F32, kind="ExternalOutput").ap()

    X = mybir.AxisListType.X
    Alu = mybir.AluOpType

    with TileContext(nc) as tc:
        with (
            tc.tile_pool(name="wpool", bufs=1) as wpool,
            tc.tile_pool(name="xpool", bufs=4) as xpool,
            tc.tile_pool(name="spool", bufs=2) as spool,
            tc.tile_pool(name="small", bufs=2) as small,
            tc.tile_pool(name="psum", bufs=4, space="PSUM") as psum_pool,
        ):
            bias_sb = wpool.tile([128, E], F32, tag="bias")
            # weight chunks as separate tiles (fine-grained deps: a matmul on
            # k-tile k waits only on its own chunk, so tile-0 matmuls start
            # as soon as chunk 0 + xh0 land). Chunks split across the two
            # HWDGE queues, issued after tile-0's x loads; low-k chunks
            # (needed first) lead each queue.
            WCHUNK = 7
            wc = [
                wpool.tile([128, WCHUNK, 2 * E], F16, tag=f"wc{c}", name=f"wc{c}")
                for c in range(KT // WCHUNK)
            ]
            def wcat_k(k):
                return wc[k // WCHUNK][:, k % WCHUNK, :]

            for tt in range(N_TILES):
                t0 = tt * 128
                xh_t = xpool.tile([128, KT, 128], F16, tag="xh")
                xl_t = xpool.tile([128, KT, 128], F16, tag="xl")
                # two different engine queues so both transfers run in parallel
                if tt == 0:
                    # Ramp orchestration. Both HWDGE queues sustain ~210GB/s
                    # only when SWDGE stays quiet, so ALL weight chunks ride
                    # sync/scalar, interleaved with quarter-chunks of tile-0's
                    # x in exact need-order: MM0 gates on just xh0q0+wc0
                    # (~1.4MB), and every later matmul's chunk lands barely
                    # ahead of its first use.
                    def wdma(eng, c):
                        eng.dma_start(wc[c][:], wcat_d[:, c, :, :])

                    XC = KT // 4  # 14
                    # sync: xh0q0 wc0 xh0q1 wc2 xh0q2 wc4 xh0q3 wc6
                    for j in range(4):
                        nc.sync.dma_start(
                            xh_t[:, j * XC : (j + 1) * XC, :],
                            xh_d[:, 0, j * XC : (j + 1) * XC, :],
                        )
                        wdma(nc.sync, 2 * j)
                    # scalar: wc1 xl0q0 wc3 xl0q1 wc5 xl0q2 wc7 xl0q3
                    for j in range(4):
                        wdma(nc.scalar, 2 * j + 1)
                        nc.scalar.dma_start(
                            xl_t[:, j * XC : (j + 1) * XC, :],
                            xl_d[:, 0, j * XC : (j + 1) * XC, :],
                        )
                    # bias last on scalar HWDGE: keeps SWDGE fully out of the
                    # ramp (SWDGE traffic starves the HWDGE queues) and bias
                    # isn't needed until the first routing chain (~35us)
                    nc.scalar.dma_start(bias_sb[:], bias_d)
                elif tt <= 2:
                    # halves: progressive consumption while the queues drain
                    # the ramp backlog
                    XH2 = KT // 2
                    for j in range(2):
                        nc.sync.dma_start(
                            xh_t[:, j * XH2 : (j + 1) * XH2, :],
                            xh_d[:, tt, j * XH2 : (j + 1) * XH2, :],
                        )
                        nc.scalar.dma_start(
                            xl_t[:, j * XH2 : (j + 1) * XH2, :],
                            xl_d[:, tt, j * XH2 : (j + 1) * XH2, :],
                        )
                else:
                    nc.sync.dma_start(xh_t[:], xh_d[:, tt, :, :])
                    nc.scalar.dma_start(xl_t[:], xl_d[:, tt, :, :])

                # psum [128, 512]: cols 0:256 accumulate xh@wh + xl@wh,
                # cols 256:512 accumulate xh@wl. logits = left + right halves.
                ps = psum_pool.tile([128, 2 * E], F32, tag="ps")
                for k in range(KT):
                    nc.tensor.matmul(
                        ps[:], xh_t[:, k, :], wcat_k(k),
                        start=(k == 0), stop=False, skip_group_check=True,
                    )
                for k in range(KT):
                    nc.tensor.matmul(
                        ps[:, 0:E], xl_t[:, k, :], wcat_k(k)[:, 0:E],
                        start=False, stop=(k == KT - 1), skip_group_check=True,
                    )

                # logits*1024 = ps_left + ps_right; then sigmoid with 1/1024
                # (TT can read only one PSUM operand: bounce right half via ACT)
                psr = spool.tile([128, E], F32, tag="scratch")
                nc.scalar.activation(
                    psr[:], ps[:, E : 2 * E], mybir.ActivationFunctionType.Copy
                )
                lg = spool.tile([128, E], F32, tag="lg")
                nc.vector.tensor_add(lg[:], ps[:, 0:E], psr[:])
                scores = spool.tile([128, E], F32, tag="scores")
                nc.scalar.activation(
                    scores[:], lg[:],
                    mybir.ActivationFunctionType.Sigmoid,
                    scale=1.0 / W_SCALE,
                )

                # scores_for_choice = scores + bias  (bias varies along free dim)
                sfc = spool.tile([128, E], F32, tag="sfc")
                nc.vector.tensor_add(sfc[:], scores[:], bias_sb[:])

                # per-group top-2 sum: g1 = grouped max, remove it, g2 = grouped max
                sfc_g = sfc[:].rearrange("p (g e) -> p g e", g=N_GROUP)
                g1 = small.tile([128, N_GROUP], F32, tag="g1")
                nc.vector.reduce_max(g1[:], sfc_g, axis=X)
                sfc_mr = spool.tile([128, E], F32, tag="scratch", name="sfc_mr")
                nc.vector.match_replace(sfc_mr[:], g1[:], sfc[:], NEG_BIG)
                g2 = small.tile([128, N_GROUP], F32, tag="g2")
                nc.vector.reduce_max(
                    g2[:], sfc_mr[:].rearrange("p (g e) -> p g e", g=N_GROUP), axis=X
                )
                gs = small.tile([128, N_GROUP], F32, tag="gs")
                nc.vector.tensor_add(gs[:], g1[:], g2[:])

                # top-4 groups: tau = 4th largest group score -> 0/1 mask
                gsrt = small.tile([128, 8], F32, tag="gsrt")
                nc.vector.max(out=gsrt[:], in_=gs[:])
                gmask = small.tile([128, N_GROUP], F32, tag="gmask")
                nc.vector.tensor_scalar(
                    gmask[:], gs[:], gsrt[:, TOPK_GROUP - 1 : TOPK_GROUP], None,
                    op0=Alu.is_ge,
                )

                # tmp = sfc * mask (expanded over the 32 experts of each group)
                tmp = spool.tile([128, E], F32, tag="tmp")
                nc.vector.tensor_mul(
                    tmp[:].rearrange("p (g e) -> p g e", g=N_GROUP),
                    sfc_g,
                    gmask[:].unsqueeze(2).to_broadcast([128, N_GROUP, GROUP_SIZE]),
                )

                # ordered top-8 of tmp (+ indices, jax tie order)
                v8 = small.tile([128, 8], F32, tag="v8")
                nc.vector.max(out=v8[:], in_=tmp[:])
                i8 = small.tile([128, 8], U32, tag="i8")
                nc.vector.max_index(i8[:], v8[:], tmp[:])

                # mark the selected positions, pull raw sigmoid scores there
                tmp_mr = spool.tile([128, E], F32, tag="scratch", name="tmp_mr")
                nc.vector.match_replace(tmp_mr[:], v8[:], tmp[:], NEG_BIG)
                sel = spool.tile([128, E], F32, tag="sel")
                nc.vector.tensor_scalar(
                    sel[:], tmp_mr[:], NEG_BIG, None, op0=Alu.is_equal
                )
                scsel = spool.tile([128, E], F32, tag="scsel")
                nc.vector.tensor_mul(scsel[:], scores[:], sel[:])
                s8 = small.tile([128, 8], F32, tag="s8")
                nc.vector.max(out=s8[:], in_=scsel[:])
                s8i = small.tile([128, 8], U32, tag="s8i")
                nc.vector.max_index(s8i[:], s8[:], scsel[:])

                # idx output is ready now — fire its DMA before the w-path
                idx_out = small.tile([128, TOP_K], I32, tag="idx_out")
                nc.vector.tensor_copy(idx_out[:], i8[:])
                nc.sync.dma_start(idx_d[t0 : t0 + 128, :], idx_out[:])

                # re-pair score values to sfc order: w8[k] = sum_j s8[j]*(s8i[j]==i8[k])
                e8 = small.tile([128, 8, 8], F32, tag="e8")
                nc.vector.tensor_tensor(
                    e8[:],
                    s8i[:].unsqueeze(1).to_broadcast([128, 8, 8]),
                    i8[:].unsqueeze(2).to_broadcast([128, 8, 8]),
                    op=Alu.is_equal,
                )
                w64 = small.tile([128, 8, 8], F32, tag="w64")
                nc.vector.tensor_mul(
                    w64[:], e8[:], s8[:].unsqueeze(1).to_broadcast([128, 8, 8])
                )
                w8 = small.tile([128, 8], F32, tag="w8")
                nc.vector.reduce_sum(w8[:], w64[:], axis=X)

                # normalize: w = w8 / sum * 2.5 (the reference's +1e-20 is
                # below fp32 ulp of the denominator, which is always >1)
                ds = small.tile([128, 1], F32, tag="ds")
                nc.vector.reduce_sum(ds[:], s8[:], axis=X)
                rcp = small.tile([128, 1], F32, tag="rcp")
                nc.vector.reciprocal(rcp[:], ds[:])
                w_out = small.tile([128, TOP_K], F32, tag="w_out")
                nc.vector.tensor_scalar(
                    w_out[:], w8[:], rcp[:, 0:1], ROUTED_SCALING,
                    op0=Alu.mult, op1=Alu.mult,
                )
                nc.sync.dma_start(w_d[t0 : t0 + 128, :], w_out[:])

    nc.compile()
    return nc


_NC_CACHE = None


def _get_nc():
    global _NC_CACHE
    if _NC_CACHE is None:
        _NC_CACHE = _build_nc()
    return _NC_CACHE


def _prep_inputs(hidden_states, weight, e_score_correction_bias):
    x = np.ascontiguousarray(hidden_states, dtype=np.float32).reshape(T_FULL, H)
    wT = np.ascontiguousarray(weight, dtype=np.float32).T * W_SCALE  # [H, E]
    wh = wT.astype(np.float16)
    wl = (wT - wh.astype(np.float32)).astype(np.float16)
    wcat = np.concatenate([wh, wl], axis=1)  # [H, 2E]
    # shuffle to [p, chunk, k_in_chunk, e]: row c*7*128 + j*128 + p -> [p,c,j,e]
    wcat = np.ascontiguousarray(
        wcat.reshape(8, 7, 128, 2 * E).transpose(2, 0, 1, 3)
    )
    bias_b = np.ascontiguousarray(
        np.broadcast_to(
            np.asarray(e_score_correction_bias, dtype=np.float32)[None, :], (128, E)
        )
    )
    in_maps = []
    for c in range(N_CORES):
        xc = x[c * T_CORE : (c + 1) * T_CORE]  # [Tc, H] contiguous
        xh = xc.astype(np.float16)
        xl = (xc - xh.astype(np.float32)).astype(np.float16)
        # device layout [p, tile, k, t]: x[tt*128+t, k*128+p] -> A[p, tt, k, t]
        xh_dev = np.ascontiguousarray(
            xh.reshape(N_TILES, 128, KT, 128).transpose(3, 0, 2, 1)
        )
        xl_dev = np.ascontiguousarray(
            xl.reshape(N_TILES, 128, KT, 128).transpose(3, 0, 2, 1)
        )
        in_maps.append(
            {"xh": xh_dev, "xl": xl_dev, "wcat": wcat, "biasb": bias_b}
        )
    return in_maps


def run(hidden_states, weight, e_score_correction_bias, trace=False, **spmd_kwargs):
    nc = _get_nc()
    in_maps = _prep_inputs(hidden_states, weight, e_score_correction_bias)
    res = run_bass_kernel_spmd(
        nc, in_maps, core_ids=list(range(N_CORES)), trace=trace, **spmd_kwargs
    )
    idx = np.concatenate([r["out_idx"] for r in res.results], axis=0)
    w = np.concatenate([r["out_w"] for r in res.results], axis=0)
    return (idx.astype(np.int32), w.astype(np.float32)), res


def kernel(hidden_states, weight, e_score_correction_bias):
    (idx, w), _ = run(hidden_states, weight, e_score_correction_bias, trace=False)
    return idx, w



# revision 2
# speedup vs baseline: 1.5190x; 1.5190x over previous
"""DeepseekVL2 MoE gate (sigmoid + grouped top-k routing) on 8 trn2 cores.

Contract: kernel(**inputs) takes the FULL unsharded inputs
  hidden_states [4, 4096, 7168] f32, weight [256, 7168] f32,
  e_score_correction_bias [256] f32
and returns (topk_idx [16384, 8] int32, topk_weight [16384, 8] f32),
matching reference jax semantics.

Strategy:
  - Data parallel: 16384 tokens -> 2048 per core x 8 cores.
  - Gating GEMM as fp16 main pass + two fp8(e4m3) DoubleRow correction
    passes (2.0 fp16-units of PE work vs 3.0 for the fp16 hi/lo 3-pass):
      logits*1024 = xh@whs + (xl*128)@(whs/128) + (x/8)@(wl*8)
    where xh=fp16(x), xl=x-xh, whs=fp16(w.T*1024), wl=w.T*1024-whs.
    The fp8 plane scales are chosen so each product lands in the same
    *1024 logit scale, so all three passes accumulate into one PSUM
    bank. fp8 corrections give ~4 extra mantissa bits per operand
    (~1e-5 logit error; emulated: 3/16384 token mismatches, rel 3.8e-3).
    DoubleRow packs K=256 per fp8 matmul at 2x PE rate.
  - Routing per 128-token tile entirely on-chip with DVE max8 /
    max_index / match_replace ops (tie semantics match jax top_k).
"""

import numpy as np
import ml_dtypes

import concourse.bacc as bacc
import concourse.bass as bass
import concourse.mybir as mybir
from concourse.bass_utils import run_bass_kernel_spmd
from concourse.tile import TileContext

F16 = mybir.dt.float16
F32 = mybir.dt.float32
F8 = mybir.dt.float8e4
U32 = mybir.dt.uint32
I32 = mybir.dt.int32
E4NP = ml_dtypes.float8_e4m3

N_CORES = 8
T_FULL = 16384
T_CORE = T_FULL // N_CORES          # 2048
H = 7168
E = 256
KT = H // 128                        # 56 contraction tiles
NPAIR = KT // 2                      # 28 fp8 DoubleRow k-pairs per pass
N_TILES = T_CORE // 128              # 16 token tiles per core
N_GROUP = 8
GROUP_SIZE = E // N_GROUP            # 32
TOPK_GROUP = 4
TOP_K = 8
ROUTED_SCALING = 2.5
W_SCALE = 1024.0                     # keeps wl fp16-normal
XL_S = 128.0                         # xl plane stored as e4m3(xl*128)
X8_S = 0.125                         # x plane stored as e4m3(x/8)
NEG_BIG = -1.0e30


def _build_nc():
    nc = bacc.Bacc(
        "TRN2",
        target_bir_lowering=False,
        debug=False,
        num_devices=N_CORES,
    )

    # x planes arrive pre-shuffled to SBUF layout: [p, tile, k, t] so each
    # partition's per-tile data is one contiguous run (fast DMA). x8c packs
    # the two fp8 planes along k: k 0..55 = xl8, k 56..111 = x8.
    xh_d = nc.dram_tensor("xh", [128, N_TILES, KT, 128], F16, kind="ExternalInput").ap()
    x8c_d = nc.dram_tensor("x8c", [128, N_TILES, 2 * KT, 128], F8, kind="ExternalInput").ap()
    # w16 pre-shuffled to [p, chunk, k, e]; w8 holds DoubleRow k-pairs
    # [p, chunk, pair, ko, e] with pairs 0..27 = wh8c, 28..55 = wl8.
    w16_d = nc.dram_tensor("w16", [128, 8, 7, E], F16, kind="ExternalInput").ap()
    w8_d = nc.dram_tensor("w8", [128, 4, 14, 2, E], F8, kind="ExternalInput").ap()
    bias_d = nc.dram_tensor("biasb", [128, E], F32, kind="ExternalInput").ap()
    idx_d = nc.dram_tensor("out_idx", [T_CORE, TOP_K], I32, kind="ExternalOutput").ap()
    w_d = nc.dram_tensor("out_w", [T_CORE, TOP_K], F32, kind="ExternalOutput").ap()

    X = mybir.AxisListType.X
    Alu = mybir.AluOpType
    DR = mybir.MatmulPerfMode.DoubleRow

    with TileContext(nc) as tc:
        with (
            tc.tile_pool(name="wpool", bufs=1) as wpool,
            tc.tile_pool(name="xpool", bufs=3) as xpool,
            tc.tile_pool(name="spool", bufs=2) as spool,
            tc.tile_pool(name="small", bufs=2) as small,
            tc.tile_pool(name="psum", bufs=4, space="PSUM") as psum_pool,
        ):
            bias_sb = wpool.tile([128, E], F32, tag="bias")
            # weight chunks as separate tiles (fine-grained deps: a matmul on
            # k-tile k waits only on its own chunk, so tile-0 matmuls start
            # as soon as chunk 0 + xh0's first quarter land).
            w16c = [
                wpool.tile([128, 7, E], F16, tag=f"w16c{c}", name=f"w16c{c}")
                for c in range(8)
            ]
            w8c = [
                wpool.tile([128, 14, 2, E], F8, tag=f"w8c{c}", name=f"w8c{c}")
                for c in range(4)
            ]

            def w16_k(k):
                return w16c[k // 7][:, k % 7, :]

            def w8_pair(j):  # j in 0..55 (28 wh8c pairs then 28 wl8 pairs)
                return w8c[j // 14][:, j % 14, :, :]

            for tt in range(N_TILES):
                t0 = tt * 128
                xh_t = xpool.tile([128, KT, 128], F16, tag="xh")
                x8_t = xpool.tile([128, 2 * KT, 128], F8, tag="x8")
                # two different engine queues so both transfers run in
                # parallel: sync carries xh + w16, scalar carries x8c + w8.
                if tt == 0:
                    # Ramp orchestration: interleave tile-0's x quarters with
                    # weight chunks in need-order so MM0 gates on just
                    # xh0q0+w16c0 and later matmuls' data lands just ahead
                    # of first use.
                    XC = KT // 4  # 14
                    for j in range(4):
                        nc.sync.dma_start(
                            xh_t[:, j * XC : (j + 1) * XC, :],
                            xh_d[:, 0, j * XC : (j + 1) * XC, :],
                        )
                        nc.sync.dma_start(w16c[2 * j][:], w16_d[:, 2 * j, :, :])
                        nc.sync.dma_start(w16c[2 * j + 1][:], w16_d[:, 2 * j + 1, :, :])
                    X2 = KT // 2  # 28
                    for j in range(2):
                        nc.scalar.dma_start(
                            x8_t[:, j * X2 : (j + 1) * X2, :],
                            x8c_d[:, 0, j * X2 : (j + 1) * X2, :],
                        )
                        nc.scalar.dma_start(w8c[j][:], w8_d[:, j, :, :, :])
                    for j in range(2):
                        nc.scalar.dma_start(
                            x8_t[:, (2 + j) * X2 : (3 + j) * X2, :],
                            x8c_d[:, 0, (2 + j) * X2 : (3 + j) * X2, :],
                        )
                        nc.scalar.dma_start(w8c[2 + j][:], w8_d[:, 2 + j, :, :, :])
                    # bias last: not needed until the first routing chain
                    nc.sync.dma_start(bias_sb[:], bias_d)
                elif tt <= 2:
                    # halves: progressive consumption while the queues drain
                    # the ramp backlog
                    XH2 = KT // 2
                    for j in range(2):
                        nc.sync.dma_start(
                            xh_t[:, j * XH2 : (j + 1) * XH2, :],
                            xh_d[:, tt, j * XH2 : (j + 1) * XH2, :],
                        )
                        nc.scalar.dma_start(
                            x8_t[:, 2 * j * XH2 : 2 * (j + 1) * XH2, :],
                            x8c_d[:, tt, 2 * j * XH2 : 2 * (j + 1) * XH2, :],
                        )
                else:
                    nc.sync.dma_start(xh_t[:], xh_d[:, tt, :, :])
                    nc.scalar.dma_start(x8_t[:], x8c_d[:, tt, :, :])

                # single PSUM accumulator [128, 256]: fp16 main pass then the
                # two fp8 DoubleRow correction passes (same logit scale).
                ps = psum_pool.tile([128, E], F32, tag="ps")
                for k in range(KT):
                    nc.tensor.matmul(
                        ps[:], xh_t[:, k, :], w16_k(k),
                        start=(k == 0), stop=False, skip_group_check=True,
                    )
                for j in range(NPAIR):  # xl8 @ wh8c
                    nc.tensor.matmul(
                        ps[:], x8_t[:, 2 * j : 2 * j + 2, :], w8_pair(j),
                        start=False, stop=False, perf_mode=DR,
                        skip_group_check=True,
                    )
                for j in range(NPAIR):  # x8 @ wl8
                    nc.tensor.matmul(
                        ps[:], x8_t[:, KT + 2 * j : KT + 2 * j + 2, :],
                        w8_pair(NPAIR + j),
                        start=False, stop=(j == NPAIR - 1), perf_mode=DR,
                        skip_group_check=True,
                    )

                # scores = sigmoid(logits) with the 1/1024 scale folded in
                scores = spool.tile([128, E], F32, tag="scores")
                nc.scalar.activation(
                    scores[:], ps[:],
                    mybir.ActivationFunctionType.Sigmoid,
                    scale=1.0 / W_SCALE,
                )

                # scores_for_choice = scores + bias  (bias varies along free dim)
                sfc = spool.tile([128, E], F32, tag="sfc")
                nc.vector.tensor_add(sfc[:], scores[:], bias_sb[:])

                # per-group top-2 sum: g1 = grouped max, remove it, g2 = grouped max
                sfc_g = sfc[:].rearrange("p (g e) -> p g e", g=N_GROUP)
                g1 = small.tile([128, N_GROUP], F32, tag="g1")
                nc.vector.reduce_max(g1[:], sfc_g, axis=X)
                sfc_mr = spool.tile([128, E], F32, tag="scratch", name="sfc_mr")
                nc.vector.match_replace(sfc_mr[:], g1[:], sfc[:], NEG_BIG)
                g2 = small.tile([128, N_GROUP], F32, tag="g2")
                nc.vector.reduce_max(
                    g2[:], sfc_mr[:].rearrange("p (g e) -> p g e", g=N_GROUP), axis=X
                )
                gs = small.tile([128, N_GROUP], F32, tag="gs")
                nc.vector.tensor_add(gs[:], g1[:], g2[:])

                # top-4 groups: tau = 4th largest group score -> 0/1 mask
                gsrt = small.tile([128, 8], F32, tag="gsrt")
                nc.vector.max(out=gsrt[:], in_=gs[:])
                gmask = small.tile([128, N_GROUP], F32, tag="gmask")
                nc.vector.tensor_scalar(
                    gmask[:], gs[:], gsrt[:, TOPK_GROUP - 1 : TOPK_GROUP], None,
                    op0=Alu.is_ge,
                )

                # tmp = sfc * mask (expanded over the 32 experts of each group)
                tmp = spool.tile([128, E], F32, tag="tmp")
                nc.vector.tensor_mul(
                    tmp[:].rearrange("p (g e) -> p g e", g=N_GROUP),
                    sfc_g,
                    gmask[:].unsqueeze(2).to_broadcast([128, N_GROUP, GROUP_SIZE]),
                )

                # ordered top-8 of tmp (+ indices, jax tie order)
                v8 = small.tile([128, 8], F32, tag="v8")
                nc.vector.max(out=v8[:], in_=tmp[:])
                i8 = small.tile([128, 8], U32, tag="i8")
                nc.vector.max_index(i8[:], v8[:], tmp[:])

                # mark the selected positions, pull raw sigmoid scores there
                tmp_mr = spool.tile([128, E], F32, tag="scratch", name="tmp_mr")
                nc.vector.match_replace(tmp_mr[:], v8[:], tmp[:], NEG_BIG)
                sel = spool.tile([128, E], F32, tag="sel")
                nc.vector.tensor_scalar(
                    sel[:], tmp_mr[:], NEG_BIG, None, op0=Alu.is_equal
                )
                scsel = spool.tile([128, E], F32, tag="scsel")
                nc.vector.tensor_mul(scsel[:], scores[:], sel[:])
                s8 = small.tile([128, 8], F32, tag="s8")
                nc.vector.max(out=s8[:], in_=scsel[:])
                s8i = small.tile([128, 8], U32, tag="s8i")
                nc.vector.max_index(s8i[:], s8[:], scsel[:])

                # idx output is ready now — fire its DMA before the w-path
                idx_out = small.tile([128, TOP_K], I32, tag="idx_out")
                nc.vector.tensor_copy(idx_out[:], i8[:])
                nc.sync.dma_start(idx_d[t0 : t0 + 128, :], idx_out[:])

                # re-pair score values to sfc order: w8[k] = sum_j s8[j]*(s8i[j]==i8[k])
                e8 = small.tile([128, 8, 8], F32, tag="e8")
                nc.vector.tensor_tensor(
                    e8[:],
                    s8i[:].unsqueeze(1).to_broadcast([128, 8, 8]),
                    i8[:].unsqueeze(2).to_broadcast([128, 8, 8]),
                    op=Alu.is_equal,
                )
                w64 = small.tile([128, 8, 8], F32, tag="w64")
                nc.vector.tensor_mul(
                    w64[:], e8[:], s8[:].unsqueeze(1).to_broadcast([128, 8, 8])
                )
                w8v = small.tile([128, 8], F32, tag="w8v")
                nc.vector.reduce_sum(w8v[:], w64[:], axis=X)

                # normalize: w = w8v / sum * 2.5 (the reference's +1e-20 is
                # below fp32 ulp of the denominator, which is always >1)
                ds = small.tile([128, 1], F32, tag="ds")
                nc.vector.reduce_sum(ds[:], s8[:], axis=X)
                rcp = small.tile([128, 1], F32, tag="rcp")
                nc.vector.reciprocal(rcp[:], ds[:])
                w_out = small.tile([128, TOP_K], F32, tag="w_out")
                nc.vector.tensor_scalar(
                    w_out[:], w8v[:], rcp[:, 0:1], ROUTED_SCALING,
                    op0=Alu.mult, op1=Alu.mult,
                )
                nc.sync.dma_start(w_d[t0 : t0 + 128, :], w_out[:])

    nc.compile()
    return nc


_NC_CACHE = None


def _get_nc():
    global _NC_CACHE
    if _NC_CACHE is None:
        _NC_CACHE = _build_nc()
    return _NC_CACHE


def _prep_inputs(hidden_states, weight, e_score_correction_bias):
    x = np.ascontiguousarray(hidden_states, dtype=np.float32).reshape(T_FULL, H)
    wT = np.ascontiguousarray(weight, dtype=np.float32).T * W_SCALE  # [H, E]
    whs = wT.astype(np.float16)
    wl = wT - whs.astype(np.float32)
    # w16 shuffle to [p, chunk, k_in_chunk, e]
    w16_dev = np.ascontiguousarray(
        whs.reshape(8, 7, 128, E).transpose(2, 0, 1, 3)
    )
    # w8: DoubleRow pairs [p, chunk, pair, ko, e]; 28 wh8c pairs + 28 wl8
    wh8c = (whs.astype(np.float32) / XL_S).astype(E4NP)
    wl8 = (wl / X8_S).astype(E4NP)
    w8_all = np.concatenate(
        [wh8c.reshape(NPAIR, 2, 128, E), wl8.reshape(NPAIR, 2, 128, E)], axis=0
    )  # [56, 2, 128, E]
    w8_dev = np.ascontiguousarray(
        w8_all.reshape(4, 14, 2, 128, E).transpose(3, 0, 1, 2, 4)
    )
    bias_b = np.ascontiguousarray(
        np.broadcast_to(
            np.asarray(e_score_correction_bias, dtype=np.float32)[None, :], (128, E)
        )
    )
    in_maps = []
    for c in range(N_CORES):
        xc = x[c * T_CORE : (c + 1) * T_CORE]  # [Tc, H] contiguous
        xh = xc.astype(np.float16)
        xl = xc - xh.astype(np.float32)
        xl8 = (xl * XL_S).astype(E4NP)
        x8 = (xc * X8_S).astype(E4NP)
        # device layout [p, tile, k, t]: x[tt*128+t, k*128+p] -> A[p, tt, k, t]
        xh_dev = np.ascontiguousarray(
            xh.reshape(N_TILES, 128, KT, 128).transpose(3, 0, 2, 1)
        )
        x8c = np.concatenate(
            [
                xl8.reshape(N_TILES, 128, KT, 128),
                x8.reshape(N_TILES, 128, KT, 128),
            ],
            axis=2,
        )  # [tile, t, 112, p]
        x8c_dev = np.ascontiguousarray(x8c.transpose(3, 0, 2, 1))
        in_maps.append(
            {
                "xh": xh_dev,
                "x8c": x8c_dev,
                "w16": w16_dev,
                "w8": w8_dev,
                "biasb": bias_b,
            }
        )
    return in_maps


def run(hidden_states, weight, e_score_correction_bias, trace=False, **spmd_kwargs):
    nc = _get_nc()
    in_maps = _prep_inputs(hidden_states, weight, e_score_correction_bias)
    res = run_bass_kernel_spmd(
        nc, in_maps, core_ids=list(range(N_CORES)), trace=trace, **spmd_kwargs
    )
    idx = np.concatenate([r["out_idx"] for r in res.results], axis=0)
    w = np.concatenate([r["out_w"] for r in res.results], axis=0)
    return (idx.astype(np.int32), w.astype(np.float32)), res


def kernel(hidden_states, weight, e_score_correction_bias):
    (idx, w), _ = run(hidden_states, weight, e_score_correction_bias, trace=False)
    return idx, w


# revision 5
# speedup vs baseline: 1.6123x; 1.0614x over previous
"""DeepseekVL2 MoE gate (sigmoid + grouped top-k routing) on 8 trn2 cores.

Contract: kernel(**inputs) takes the FULL unsharded inputs
  hidden_states [4, 4096, 7168] f32, weight [256, 7168] f32,
  e_score_correction_bias [256] f32
and returns (topk_idx [16384, 8] int32, topk_weight [16384, 8] f32),
matching reference jax semantics.

Strategy:
  - Data parallel: 16384 tokens -> 2048 per core x 8 cores.
  - Gating GEMM as fp16 main pass + two fp8(e4m3) DoubleRow correction
    passes (2.0 fp16-units of PE work vs 3.0 for the fp16 hi/lo 3-pass):
      logits*1024 = xh@whs + (xl*128)@(whs/128) + (xh/8)@(wl*8)
    with xh=fp16(x), xl=x-xh, whs=fp16(w.T*1024), wl=w.T*1024-whs.
    Scales put every product in the same *1024 logit scale so all three
    passes accumulate into one PSUM bank. DoubleRow packs K=256 per
    fp8 matmul at 2x PE rate. Emulated: 3/16384 token mismatches.
  - DMA is the co-bottleneck (~310GB/s/core aggregate over the two
    HWDGE queues, near the HBM roof), so the (xh/8) fp8 plane is NOT
    shipped: it is derived on-chip from xh by a DVE cast that is
    pipelined one tile ahead (emitted before the previous tile's
    routing chain so it never stalls the tensor engine). Remaining
    51.6MB/core is split evenly: Q1(sync) xh k0..43 + w16,
    Q2(scalar) xh k44..55 + xl8 + w8 + bias + outputs.
  - Routing per 128-token tile entirely on-chip with DVE max8 /
    max_index / match_replace ops (tie semantics match jax top_k).
"""

import numpy as np
import ml_dtypes

import concourse.bacc as bacc
import concourse.bass as bass
import concourse.mybir as mybir
from concourse.bass_utils import run_bass_kernel_spmd
from concourse.tile import TileContext

F16 = mybir.dt.float16
F32 = mybir.dt.float32
F8 = mybir.dt.float8e4
U32 = mybir.dt.uint32
I32 = mybir.dt.int32
E4NP = ml_dtypes.float8_e4m3

N_CORES = 8
T_FULL = 16384
T_CORE = T_FULL // N_CORES          # 2048
H = 7168
E = 256
KT = H // 128                        # 56 contraction tiles
NPAIR = KT // 2                      # 28 fp8 DoubleRow k-pairs per pass
N_TILES = T_CORE // 128              # 16 token tiles per core
KSPLIT = 44                          # xh k-tiles 0..43 on Q1, 44..55 on Q2
N_GROUP = 8
GROUP_SIZE = E // N_GROUP            # 32
TOPK_GROUP = 4
TOP_K = 8
ROUTED_SCALING = 2.5
W_SCALE = 1024.0                     # keeps wl fp16-normal
XL_S = 128.0                         # xl plane stored as e4m3(xl*128)
X8_S = 0.125                         # x8 derived on-chip as e4m3(xh/8)
NEG_BIG = -1.0e30


def _build_nc():
    nc = bacc.Bacc(
        "TRN2",
        target_bir_lowering=False,
        debug=False,
        num_devices=N_CORES,
    )

    # x planes arrive pre-shuffled to SBUF layout [p, tile, k, t]
    xh_d = nc.dram_tensor("xh", [128, N_TILES, KT, 128], F16, kind="ExternalInput").ap()
    xl8_d = nc.dram_tensor("xl8", [128, N_TILES, KT, 128], F8, kind="ExternalInput").ap()
    # w16 pre-shuffled to [p, chunk, k, e]; w8 holds DoubleRow k-pairs
    # [p, chunk, pair, ko, e] with pairs 0..27 = wh8c, 28..55 = wl8.
    w16_d = nc.dram_tensor("w16", [128, 8, 7, E], F16, kind="ExternalInput").ap()
    w8_d = nc.dram_tensor("w8", [128, 4, 14, 2, E], F8, kind="ExternalInput").ap()
    bias_d = nc.dram_tensor("biasb", [128, E], F32, kind="ExternalInput").ap()
    idx_d = nc.dram_tensor("out_idx", [T_CORE, TOP_K], I32, kind="ExternalOutput").ap()
    w_d = nc.dram_tensor("out_w", [T_CORE, TOP_K], F32, kind="ExternalOutput").ap()

    X = mybir.AxisListType.X
    Alu = mybir.AluOpType
    DR = mybir.MatmulPerfMode.DoubleRow

    with TileContext(nc) as tc:
        with (
            tc.tile_pool(name="wpool", bufs=1) as wpool,
            tc.tile_pool(name="xpool", bufs=3) as xpool,
            tc.tile_pool(name="x8pool", bufs=3) as x8pool,
            tc.tile_pool(name="spool", bufs=2) as spool,
            tc.tile_pool(name="small", bufs=2) as small,
            tc.tile_pool(name="psum", bufs=4, space="PSUM") as psum_pool,
        ):
            bias_sb = wpool.tile([128, E], F32, tag="bias")
            w16c = [
                wpool.tile([128, 7, E], F16, tag=f"w16c{c}", name=f"w16c{c}")
                for c in range(8)
            ]
            w8c = [
                wpool.tile([128, 14, 2, E], F8, tag=f"w8c{c}", name=f"w8c{c}")
                for c in range(4)
            ]

            def w16_k(k):
                return w16c[k // 7][:, k % 7, :]

            def w8_pair(j):  # j in 0..55 (28 wh8c pairs then 28 wl8 pairs)
                return w8c[j // 14][:, j % 14, :, :]

            def load_tile(tt, xh_t, xl8_t):
                if tt == 0:
                    # Ramp: interleave tile-0 x pieces with weight chunks in
                    # first-use order. Q1 feeds the fp16 pass, Q2 the fp8.
                    bnd = [0, 7, 14, 21, 28, KSPLIT]
                    for j in range(5):
                        nc.sync.dma_start(
                            xh_t[:, bnd[j] : bnd[j + 1], :],
                            xh_d[:, 0, bnd[j] : bnd[j + 1], :],
                        )
                        if j < 4:
                            nc.sync.dma_start(w16c[j][:], w16_d[:, j, :, :])
                    for c in range(4, 8):
                        nc.sync.dma_start(w16c[c][:], w16_d[:, c, :, :])
                    nc.scalar.dma_start(
                        xh_t[:, KSPLIT:KT, :], xh_d[:, 0, KSPLIT:KT, :]
                    )
                    nc.scalar.dma_start(xl8_t[:, 0:28, :], xl8_d[:, 0, 0:28, :])
                    nc.scalar.dma_start(w8c[0][:], w8_d[:, 0, :, :, :])
                    nc.scalar.dma_start(xl8_t[:, 28:KT, :], xl8_d[:, 0, 28:KT, :])
                    nc.scalar.dma_start(w8c[1][:], w8_d[:, 1, :, :, :])
                    nc.scalar.dma_start(w8c[2][:], w8_d[:, 2, :, :, :])
                    nc.scalar.dma_start(w8c[3][:], w8_d[:, 3, :, :, :])
                    nc.scalar.dma_start(bias_sb[:], bias_d)
                elif tt <= 2:
                    # halves: progressive consumption while the queues drain
                    # the ramp backlog
                    nc.sync.dma_start(
                        xh_t[:, 0:22, :], xh_d[:, tt, 0:22, :]
                    )
                    nc.sync.dma_start(
                        xh_t[:, 22:KSPLIT, :], xh_d[:, tt, 22:KSPLIT, :]
                    )
                    nc.scalar.dma_start(
                        xh_t[:, KSPLIT:KT, :], xh_d[:, tt, KSPLIT:KT, :]
                    )
                    for j in range(2):
                        nc.scalar.dma_start(
                            xl8_t[:, j * 28 : (j + 1) * 28, :],
                            xl8_d[:, tt, j * 28 : (j + 1) * 28, :],
                        )
                else:
                    nc.sync.dma_start(
                        xh_t[:, 0:KSPLIT, :], xh_d[:, tt, 0:KSPLIT, :]
                    )
                    nc.scalar.dma_start(
                        xh_t[:, KSPLIT:KT, :], xh_d[:, tt, KSPLIT:KT, :]
                    )
                    nc.scalar.dma_start(xl8_t[:], xl8_d[:, tt, :, :])

            # prologue: tile 0 loads + its on-chip x8 derivation
            xh_tiles = {}
            xl8_tiles = {}
            x8_tiles = {}
            xh_tiles[0] = xpool.tile([128, KT, 128], F16, tag="xh", name="xh0")
            xl8_tiles[0] = xpool.tile([128, KT, 128], F8, tag="xl8", name="xl80")
            load_tile(0, xh_tiles[0], xl8_tiles[0])
            x8_tiles[0] = x8pool.tile([128, KT, 128], F8, tag="x8", name="x80")
            nc.vector.tensor_scalar(
                x8_tiles[0][:], xh_tiles[0][:], X8_S, None, op0=Alu.mult
            )

            for tt in range(N_TILES):
                t0 = tt * 128
                xh_t, xl8_t, x8_t = xh_tiles[tt], xl8_tiles[tt], x8_tiles[tt]
                # prefetch DMAs for the next tile (engine queues run ahead)
                if tt + 1 < N_TILES:
                    xh_tiles[tt + 1] = xpool.tile([128, KT, 128], F16, tag="xh", name=f"xh{tt+1}")
                    xl8_tiles[tt + 1] = xpool.tile([128, KT, 128], F8, tag="xl8", name=f"xl8{tt+1}")
                    load_tile(tt + 1, xh_tiles[tt + 1], xl8_tiles[tt + 1])

                # PSUM accumulator [128, 256]: fp16 main pass + two fp8
                # DoubleRow correction passes, all in the same logit scale.
                ps = psum_pool.tile([128, E], F32, tag="ps")
                for k in range(KT):
                    nc.tensor.matmul(
                        ps[:], xh_t[:, k, :], w16_k(k),
                        start=(k == 0), stop=False, skip_group_check=True,
                    )
                for j in range(NPAIR):  # xl8 @ wh8c
                    nc.tensor.matmul(
                        ps[:], xl8_t[:, 2 * j : 2 * j + 2, :], w8_pair(j),
                        start=False, stop=False, perf_mode=DR,
                        skip_group_check=True,
                    )
                for j in range(NPAIR):  # x8 @ wl8
                    nc.tensor.matmul(
                        ps[:], x8_t[:, 2 * j : 2 * j + 2, :],
                        w8_pair(NPAIR + j),
                        start=False, stop=(j == NPAIR - 1), perf_mode=DR,
                        skip_group_check=True,
                    )

                # scores = sigmoid(logits) with the 1/1024 scale folded in
                scores = spool.tile([128, E], F32, tag="scores")
                nc.scalar.activation(
                    scores[:], ps[:],
                    mybir.ActivationFunctionType.Sigmoid,
                    scale=1.0 / W_SCALE,
                )

                # next tile's x8 derivation goes on the DVE queue BEFORE this
                # tile's routing chain, so it completes well before the next
                # tile's fp8 w-correction matmuls need it.
                if tt + 1 < N_TILES:
                    x8_tiles[tt + 1] = x8pool.tile([128, KT, 128], F8, tag="x8", name=f"x8{tt+1}")
                    nc.vector.tensor_scalar(
                        x8_tiles[tt + 1][:], xh_tiles[tt + 1][:], X8_S, None,
                        op0=Alu.mult,
                    )

                # scores_for_choice = scores + bias  (bias varies along free dim)
                sfc = spool.tile([128, E], F32, tag="sfc")
                nc.vector.tensor_add(sfc[:], scores[:], bias_sb[:])

                # per-group top-2 sum: g1 = grouped max, remove it, g2 = grouped max
                sfc_g = sfc[:].rearrange("p (g e) -> p g e", g=N_GROUP)
                g1 = small.tile([128, N_GROUP], F32, tag="g1")
                nc.vector.reduce_max(g1[:], sfc_g, axis=X)
                sfc_mr = spool.tile([128, E], F32, tag="scratch", name="sfc_mr")
                nc.vector.match_replace(sfc_mr[:], g1[:], sfc[:], NEG_BIG)
                g2 = small.tile([128, N_GROUP], F32, tag="g2")
                nc.vector.reduce_max(
                    g2[:], sfc_mr[:].rearrange("p (g e) -> p g e", g=N_GROUP), axis=X
                )
                gs = small.tile([128, N_GROUP], F32, tag="gs")
                nc.vector.tensor_add(gs[:], g1[:], g2[:])

                # top-4 groups: tau = 4th largest group score -> 0/1 mask
                gsrt = small.tile([128, 8], F32, tag="gsrt")
                nc.vector.max(out=gsrt[:], in_=gs[:])
                gmask = small.tile([128, N_GROUP], F32, tag="gmask")
                nc.vector.tensor_scalar(
                    gmask[:], gs[:], gsrt[:, TOPK_GROUP - 1 : TOPK_GROUP], None,
                    op0=Alu.is_ge,
                )

                # tmp = sfc * mask (expanded over the 32 experts of each group)
                tmp = spool.tile([128, E], F32, tag="tmp")
                nc.vector.tensor_mul(
                    tmp[:].rearrange("p (g e) -> p g e", g=N_GROUP),
                    sfc_g,
                    gmask[:].unsqueeze(2).to_broadcast([128, N_GROUP, GROUP_SIZE]),
                )

                # ordered top-8 of tmp (+ indices, jax tie order)
                v8 = small.tile([128, 8], F32, tag="v8")
                nc.vector.max(out=v8[:], in_=tmp[:])
                i8 = small.tile([128, 8], U32, tag="i8")
                nc.vector.max_index(i8[:], v8[:], tmp[:])

                # mark the selected positions, pull raw sigmoid scores there
                tmp_mr = spool.tile([128, E], F32, tag="scratch", name="tmp_mr")
                nc.vector.match_replace(tmp_mr[:], v8[:], tmp[:], NEG_BIG)
                sel = spool.tile([128, E], F32, tag="sel")
                nc.vector.tensor_scalar(
                    sel[:], tmp_mr[:], NEG_BIG, None, op0=Alu.is_equal
                )
                scsel = spool.tile([128, E], F32, tag="scsel")
                nc.vector.tensor_mul(scsel[:], scores[:], sel[:])
                s8 = small.tile([128, 8], F32, tag="s8")
                nc.vector.max(out=s8[:], in_=scsel[:])
                s8i = small.tile([128, 8], U32, tag="s8i")
                nc.vector.max_index(s8i[:], s8[:], scsel[:])

                # idx output is ready now — fire its DMA before the w-path
                idx_out = small.tile([128, TOP_K], I32, tag="idx_out")
                nc.vector.tensor_copy(idx_out[:], i8[:])
                nc.scalar.dma_start(idx_d[t0 : t0 + 128, :], idx_out[:])

                # re-pair score values to sfc order: w8[k] = sum_j s8[j]*(s8i[j]==i8[k])
                e8 = small.tile([128, 8, 8], F32, tag="e8")
                nc.vector.tensor_tensor(
                    e8[:],
                    s8i[:].unsqueeze(1).to_broadcast([128, 8, 8]),
                    i8[:].unsqueeze(2).to_broadcast([128, 8, 8]),
                    op=Alu.is_equal,
                )
                w64 = small.tile([128, 8, 8], F32, tag="w64")
                nc.vector.tensor_mul(
                    w64[:], e8[:], s8[:].unsqueeze(1).to_broadcast([128, 8, 8])
                )
                w8v = small.tile([128, 8], F32, tag="w8v")
                nc.vector.reduce_sum(w8v[:], w64[:], axis=X)

                # normalize: w = w8v / sum * 2.5 (the reference's +1e-20 is
                # below fp32 ulp of the denominator, which is always >1)
                ds = small.tile([128, 1], F32, tag="ds")
                nc.vector.reduce_sum(ds[:], s8[:], axis=X)
                rcp = small.tile([128, 1], F32, tag="rcp")
                nc.vector.reciprocal(rcp[:], ds[:])
                w_out = small.tile([128, TOP_K], F32, tag="w_out")
                nc.vector.tensor_scalar(
                    w_out[:], w8v[:], rcp[:, 0:1], ROUTED_SCALING,
                    op0=Alu.mult, op1=Alu.mult,
                )
                nc.scalar.dma_start(w_d[t0 : t0 + 128, :], w_out[:])

    nc.compile()
    return nc


_NC_CACHE = None


def _get_nc():
    global _NC_CACHE
    if _NC_CACHE is None:
        _NC_CACHE = _build_nc()
    return _NC_CACHE


def _prep_inputs(hidden_states, weight, e_score_correction_bias):
    x = np.ascontiguousarray(hidden_states, dtype=np.float32).reshape(T_FULL, H)
    wT = np.ascontiguousarray(weight, dtype=np.float32).T * W_SCALE  # [H, E]
    whs = wT.astype(np.float16)
    wl = wT - whs.astype(np.float32)
    w16_dev = np.ascontiguousarray(
        whs.reshape(8, 7, 128, E).transpose(2, 0, 1, 3)
    )
    # w8: DoubleRow pairs [p, chunk, pair, ko, e]; 28 wh8c pairs + 28 wl8
    wh8c = (whs.astype(np.float32) / XL_S).astype(E4NP)
    wl8 = (wl / X8_S).astype(E4NP)
    w8_all = np.concatenate(
        [wh8c.reshape(NPAIR, 2, 128, E), wl8.reshape(NPAIR, 2, 128, E)], axis=0
    )  # [56, 2, 128, E]
    w8_dev = np.ascontiguousarray(
        w8_all.reshape(4, 14, 2, 128, E).transpose(3, 0, 1, 2, 4)
    )
    bias_b = np.ascontiguousarray(
        np.broadcast_to(
            np.asarray(e_score_correction_bias, dtype=np.float32)[None, :], (128, E)
        )
    )
    in_maps = []
    for c in range(N_CORES):
        xc = x[c * T_CORE : (c + 1) * T_CORE]  # [Tc, H] contiguous
        xh = xc.astype(np.float16)
        xl = xc - xh.astype(np.float32)
        xl8 = (xl * XL_S).astype(E4NP)
        # device layout [p, tile, k, t]: x[tt*128+t, k*128+p] -> A[p, tt, k, t]
        xh_dev = np.ascontiguousarray(
            xh.reshape(N_TILES, 128, KT, 128).transpose(3, 0, 2, 1)
        )
        xl8_dev = np.ascontiguousarray(
            xl8.reshape(N_TILES, 128, KT, 128).transpose(3, 0, 2, 1)
        )
        in_maps.append(
            {
                "xh": xh_dev,
                "xl8": xl8_dev,
                "w16": w16_dev,
                "w8": w8_dev,
                "biasb": bias_b,
            }
        )
    return in_maps


def run(hidden_states, weight, e_score_correction_bias, trace=False, **spmd_kwargs):
    nc = _get_nc()
    in_maps = _prep_inputs(hidden_states, weight, e_score_correction_bias)
    res = run_bass_kernel_spmd(
        nc, in_maps, core_ids=list(range(N_CORES)), trace=trace, **spmd_kwargs
    )
    idx = np.concatenate([r["out_idx"] for r in res.results], axis=0)
    w = np.concatenate([r["out_w"] for r in res.results], axis=0)
    return (idx.astype(np.int32), w.astype(np.float32)), res


def kernel(hidden_states, weight, e_score_correction_bias):
    (idx, w), _ = run(hidden_states, weight, e_score_correction_bias, trace=False)
    return idx, w
